# revision 9
# baseline (speedup 1.0000x reference)
"""Bilateral filter (B,C,H,W)=(2,3,384,384), ksize=9 on 8 Trainium2 NeuronCores.

Strategy
--------
Data-parallel over H: core k owns output rows [48k, 48k+48) for every (b, c).

Host side packs, per core, 1152 "units" (one output row-segment of 96 pixels
each) into a [128 partitions x 9 groups] SBUF-friendly slab; each unit stores
its padded 9x104 input window (reflect padding resolved on host).  A tap
(di, dj) of the 9x9 stencil is then a pure free-dim offset read of the slab.

The 9x9 taps are processed in 18 groups (di x column-parity); one DVE
instruction covers all 5 (even dj) or 4 (odd dj) taps of a group through a
3-free-dim overlapped access pattern [(taps, step 2), (9 units, 936), (96, 1)],
amortizing the per-instruction + DRAIN overhead of the vector engine.

Per-tap math (the reference's per-pixel wd normalization cancels between
numerator and denominator):

    d   = p - x                      (DVE, bf16, batched per group)
    s   = d^2                        (DVE, bf16, batched per group)
    w   = exp(-s/(2 sigma^2) + ln ws_t)   (ACT, per-tap slice, free scale+bias)
    wd  = w * d                      (DVE for even groups, GPSIMD for odd)
    num += wd ; den += w             (TensorE identity-matmul into PSUM, fp32)

    out = x_f32 + num / den          (fp32 tail)

dtype: bf16 on-chip for 2x DVE tensor_tensor throughput; accumulation and
final arithmetic in fp32 (PSUM).  Odd-dj taps read a one-element-shifted slab
copy (slabB, its own DMA) so every DVE operand stays 4-byte aligned.

The tensor engine's HAM clock gate (1.2 vs 2.4 GHz) is kept warm with an
initial junk-matmul burst plus filler matmuls interleaved with the real
accumulation matmuls.
"""

import numpy as np
import ml_dtypes

BF16 = ml_dtypes.bfloat16

B, C, H, W = 2, 3, 384, 384
KS = 9
PAD = 4
SIGMA = 0.3 * ((KS - 1) / 2.0 - 1) + 0.8  # 1.7
C2 = 2.0 * SIGMA * SIGMA                  # 5.78
NCORES = 8
HPER = H // NCORES                        # 48
WQ = 4
WSUB = W // WQ                            # 96
WPAD = WSUB + 2 * PAD                     # 104
GROUPS = 9
NPART = 128
FREE = GROUPS * WSUB                      # 864
HALF = FREE // 2                          # 432
UNIT = KS * WPAD                          # 936
SLABF = GROUPS * UNIT                     # 8424

_ax = np.arange(KS, dtype=np.float64) - (KS // 2)
_k1 = np.exp(-(_ax ** 2) / C2)
_ws = np.outer(_k1, _k1)
_ws = _ws / _ws.sum()
LOG_WS = np.log(_ws).astype(np.float32)   # [9, 9]

_CACHE = {}


def _build_nc(fillers_per_group=0, warmup_mms=10):
    """Build the single-core Bass program (SPMD across the 8 cores)."""
    from contextlib import ExitStack

    import concourse.bass as bass
    import concourse.tile as tile
    from concourse import bacc, mybir

    f32 = mybir.dt.float32
    bf16 = mybir.dt.bfloat16
    Alu = mybir.AluOpType
    Act = mybir.ActivationFunctionType

    class DedupBacc(bacc.Bacc):
        """Every matmul here uses the same identity stationary; drop the
        redundant per-matmul Ldweights the standard pipeline emits (the PE
        array keeps its weights between matmuls), moving their sem deps onto
        the following PE instruction before wait legalization."""

        def move_matmul_waits_to_ldweights(self):
            super().move_matmul_waits_to_ldweights()
            for bb in self.main_func.blocks:
                prev_key = None
                pending = None
                keep = []
                for ins in list(bb.instructions):
                    is_pe = getattr(ins, "engine", None) == self.tensor.engine
                    if isinstance(ins, mybir.InstLdweights):
                        key = str(ins.ins[0])
                        if key == prev_key:
                            pending = ins
                            continue
                        prev_key = key
                    if is_pe and pending is not None:
                        ins.merge_dependencies_from(pending)
                        pending = None
                    keep.append(ins)
                assert pending is None
                bb.instructions[:] = keep

    nc = DedupBacc("TRN2")
    xs_d = nc.dram_tensor("xs", [NPART, SLABF], bf16, kind="ExternalInput")
    xc_d = nc.dram_tensor("xc", [NPART, FREE], f32, kind="ExternalInput")
    bt_d = nc.dram_tensor("bt", [NPART, KS * KS], f32, kind="ExternalInput")
    id_d = nc.dram_tensor("ident", [NPART, NPART], bf16, kind="ExternalInput")
    y_d = nc.dram_tensor("y", [NPART, FREE], f32, kind="ExternalOutput")

    with ExitStack() as ctx:
        tc = ctx.enter_context(tile.TileContext(nc))
        singles = ctx.enter_context(tc.tile_pool(name="singles", bufs=1))
        tapp = ctx.enter_context(tc.tile_pool(name="tapp", bufs=2))
        psum = ctx.enter_context(tc.tile_pool(name="psum", bufs=1, space="PSUM"))
        fin = ctx.enter_context(tc.tile_pool(name="fin", bufs=1))

        slabA = singles.tile([NPART, SLABF], bf16)
        slabB = singles.tile([NPART, SLABF], bf16)
        xc_sb = singles.tile([NPART, FREE], f32)
        bt_sb = singles.tile([NPART, KS * KS], f32)
        id_sb = singles.tile([NPART, NPART], bf16)

        # PE HAM warmup: junk matmuls overlapped with the slab DMA so the
        # tensor engine is at full clock when the real matmuls start.
        junk = singles.tile([NPART, 512], bf16)
        psum_scr = psum.tile([NPART, 512], f32)
        nc.vector.memset(junk[:, :], 0)
        nc.sync.dma_start(out=id_sb[:, :], in_=id_d[:, :])
        # warmup loads the identity as PE stationary; every later matmul
        # reuses it (ldweights=False), eliminating per-matmul weight reloads
        for _ in range(warmup_mms):
            nc.tensor.matmul(psum_scr[:, :], id_sb[:, :], junk[:, :],
                             start=True, stop=True)

        # split big loads across DMA queues; xc is only needed by the tail
        HSL = SLABF // 2
        nc.sync.dma_start(out=slabA[:, 0:HSL], in_=xs_d[:, 0:HSL])
        nc.sync.dma_start(out=slabA[:, HSL:SLABF], in_=xs_d[:, HSL:SLABF])
        nc.sync.dma_start(out=bt_sb[:, :], in_=bt_d[:, :])
        # one-element-shifted copy for 4B-aligned odd-dj tap reads
        nc.sync.dma_start(out=slabB[:, 0:HSL], in_=xs_d[:, 1 : HSL + 1])
        nc.sync.dma_start(out=slabB[:, HSL : SLABF - 2], in_=xs_d[:, HSL + 1 : SLABF - 1])
        nc.sync.dma_start(out=xc_sb[:, :], in_=xc_d[:, :])

        sA = slabA[:, :].rearrange("p (g r c) -> p g r c", g=GROUPS, r=KS)
        center = sA[:, :, PAD, PAD : PAD + WSUB]  # [128, 9, 96]

        # center replicated across the (max 5) tap positions of a group so the
        # batched subtract has a plain non-broadcast second operand
        center5 = singles.tile([NPART, 5, GROUPS, WSUB], bf16)
        for t in range(5):
            nc.scalar.copy(center5[:, t, :, :], center)

        num0 = psum.tile([NPART, HALF], f32)
        num1 = psum.tile([NPART, HALF], f32)
        den0 = psum.tile([NPART, HALF], f32)
        den1 = psum.tile([NPART, HALF], f32)
        started = {0: False, 1: False, 2: False, 3: False}
        nbanks = (num0, num1, den0, den1)

        # interleave parities so DVE (even-group wd) and GPSIMD (odd-group
        # wd) stay concurrently busy; lead with two even groups so the
        # shifted slabB copy has time to land
        groups = []
        for di in range(KS):
            groups.append((di, 0))
            if di >= 2:
                groups.append((di - 2, 1))
        groups += [(KS - 2, 1), (KS - 1, 1)]
        n_groups = len(groups)

        for gi, (di, par) in enumerate(groups):
            djs = [dj for dj in range(KS) if dj % 2 == par]
            nt = len(djs)
            slab = slabA if par == 0 else slabB
            base = slab[:, :]
            p_ap = bass.AP(
                tensor=base.tensor,
                offset=base.offset + di * WPAD,
                ap=[list(base.ap[0]), [2, nt], [UNIT, GROUPS], [1, WSUB]],
            )

            d5 = tapp.tile([NPART, nt, GROUPS, WSUB], bf16, tag="d5", bufs=2)
            s5 = tapp.tile([NPART, nt, GROUPS, WSUB], bf16, tag="s5", bufs=2)
            w5 = tapp.tile([NPART, nt, GROUPS, WSUB], bf16, tag="w5", bufs=2)
            wd5 = tapp.tile([NPART, nt, GROUPS, WSUB], bf16, tag="wd5", bufs=3)

            nc.vector.tensor_tensor(
                d5[:, :, :, :], p_ap, center5[:, 0:nt, :, :], Alu.subtract)
            nc.vector.tensor_tensor(
                s5[:, :, :, :], d5[:, :, :, :], d5[:, :, :, :], Alu.mult)
            for k, dj in enumerate(djs):
                tcol = di * KS + dj
                nc.scalar.activation(
                    w5[:, k, :, :], s5[:, k, :, :], Act.Exp,
                    bias=bt_sb[:, tcol : tcol + 1], scale=-1.0 / C2,
                )
            eng = nc.gpsimd if (par == 1 or di % 3 == 1) else nc.vector
            eng.tensor_tensor(
                wd5[:, :, :, :], w5[:, :, :, :], d5[:, :, :, :], Alu.mult)

            wfl = w5[:, :, :, :].rearrange("p t g c -> p (t g c)")
            wdfl = wd5[:, :, :, :].rearrange("p t g c -> p (t g c)")
            last_group = gi == n_groups - 1
            for b in range(2 * nt):
                half = b % 2
                cols = slice(b * HALF, (b + 1) * HALF)
                for bank_idx, rhs in ((half, wdfl[:, cols]),
                                      (2 + half, wfl[:, cols])):
                    tgt = nbanks[bank_idx]
                    mm = nc.tensor.matmul(
                        tgt[:, :], id_sb[:, :], rhs,
                        start=not started[bank_idx],
                        stop=last_group and b >= 2 * nt - 2,
                    )
                    mm.ins.ldweights = False
                    started[bank_idx] = True
            # keep the PE activity monitor from re-throttling the clock
            for _ in range(fillers_per_group):
                nc.tensor.matmul(psum_scr[:, :], junk[:, 0:NPART], junk[:, :],
                                 start=True, stop=True)

        y_sb = fin.tile([NPART, FREE], f32)
        for hb, (nm, dn) in enumerate(((num0, den0), (num1, den1))):
            r = fin.tile([NPART, HALF], f32, tag=f"r{hb}")
            scr = fin.tile([NPART, HALF], f32, tag=f"scr{hb}")
            nc.vector.reciprocal_approx_accurate(
                out=r[:, :], in_=dn[:, :], scratch=scr[:, :])
            t = fin.tile([NPART, HALF], f32, tag=f"t{hb}")
            nc.vector.tensor_tensor(t[:, :], nm[:, :], r[:, :], Alu.mult)
            nc.vector.tensor_tensor(
                y_sb[:, hb * HALF : (hb + 1) * HALF], t[:, :],
                xc_sb[:, hb * HALF : (hb + 1) * HALF], Alu.add)
        nc.sync.dma_start(out=y_d[:, :], in_=y_sb[:, :])

    nc.finalize()
    return nc


def get_nc():
    if "nc" not in _CACHE:
        _CACHE["nc"] = _build_nc()
    return _CACHE["nc"]


def host_shard(x):
    """x [B,C,H,W] f32 -> per-core dicts of device inputs."""
    xp = np.pad(x, ((0, 0), (0, 0), (PAD, PAD), (PAD, PAD)), mode="reflect")
    sw = np.lib.stride_tricks.sliding_window_view(xp, (KS, WPAD), axis=(2, 3))
    win = sw[:, :, :, ::WSUB]  # [B,C,384,4,9,104]
    bt = np.tile(LOG_WS.reshape(1, KS * KS), (NPART, 1)).astype(np.float32)
    ident = np.eye(NPART, dtype=BF16)
    in_maps = []
    for core in range(NCORES):
        h0 = core * HPER
        u = win[:, :, h0 : h0 + HPER].transpose(0, 1, 3, 2, 4, 5)
        slab = np.ascontiguousarray(u).reshape(NPART, SLABF).astype(BF16)
        xc = x[:, :, h0 : h0 + HPER].reshape(B, C, HPER, WQ, WSUB)
        xc = np.ascontiguousarray(xc.transpose(0, 1, 3, 2, 4))
        xc = xc.reshape(NPART, FREE).astype(np.float32)
        in_maps.append({"xs": slab, "xc": xc, "bt": bt, "ident": ident})
    return in_maps


def host_unshard(ys):
    out = np.empty((B, C, H, W), np.float32)
    for core in range(NCORES):
        h0 = core * HPER
        y = np.asarray(ys[core], np.float32).reshape(B, C, WQ, HPER, WSUB)
        out[:, :, h0 : h0 + HPER] = y.transpose(0, 1, 3, 2, 4).reshape(
            B, C, HPER, W)
    return out


def kernel(x, ksize):
    from concourse.bass_utils import run_bass_kernel_spmd

    assert int(ksize) == KS
    x = np.asarray(x, dtype=np.float32)
    assert x.shape == (B, C, H, W)
    in_maps = host_shard(x)
    nc = get_nc()
    res = run_bass_kernel_spmd(nc, in_maps, core_ids=list(range(NCORES)))
    ys = [np.asarray(r["y"]) for r in res.results]
    return host_unshard(ys)


# revision 10
# speedup vs baseline: 1.0875x; 1.0875x over previous
"""Bilateral filter (B,C,H,W)=(2,3,384,384), ksize=9 on 8 Trainium2 NeuronCores.

Strategy
--------
Data-parallel over H: core k owns output rows [48k, 48k+48) for every (b, c).

Host side packs, per core, 1152 "units" (one output row-segment of 96 pixels
each) into a [128 partitions x 9 groups] SBUF-friendly slab; each unit stores
its padded 9x104 input window (reflect padding resolved on host).  A tap
(di, dj) of the 9x9 stencil is then a pure free-dim offset read of the slab.

The 9x9 taps are processed in 18 groups (di x column-parity); one DVE
instruction covers all 5 (even dj) or 4 (odd dj) taps of a group through a
3-free-dim overlapped access pattern [(taps, step 2), (9 units, 936), (96, 1)],
amortizing the per-instruction + DRAIN overhead of the vector engine.

Per-tap math (the reference's per-pixel wd normalization cancels between
numerator and denominator):

    d   = p - x                      (DVE, bf16, batched per group)
    s   = d^2                        (DVE, bf16, batched per group)
    w   = exp(-s/(2 sigma^2) + ln ws_t)   (ACT, per-tap slice, free scale+bias)
    wd  = w * d                      (DVE for even groups, GPSIMD for odd)
    num += wd ; den += w             (TensorE identity-matmul into PSUM, fp32)

    out = x_f32 + num / den          (fp32 tail)

dtype: bf16 on-chip for 2x DVE tensor_tensor throughput; accumulation and
final arithmetic in fp32 (PSUM).  Odd-dj taps read a one-element-shifted slab
copy (slabB, its own DMA) so every DVE operand stays 4-byte aligned.

The tensor engine's HAM clock gate (1.2 vs 2.4 GHz) is kept warm with an
initial junk-matmul burst plus filler matmuls interleaved with the real
accumulation matmuls.
"""

import numpy as np
import ml_dtypes

BF16 = ml_dtypes.bfloat16

B, C, H, W = 2, 3, 384, 384
KS = 9
PAD = 4
SIGMA = 0.3 * ((KS - 1) / 2.0 - 1) + 0.8  # 1.7
C2 = 2.0 * SIGMA * SIGMA                  # 5.78
NCORES = 8
HPER = H // NCORES                        # 48
WQ = 4
WSUB = W // WQ                            # 96
WPAD = WSUB + 2 * PAD                     # 104
GROUPS = 9
NPART = 128
FREE = GROUPS * WSUB                      # 864
HALF = FREE // 2                          # 432
UNIT = KS * WPAD                          # 936
SLABF = GROUPS * UNIT                     # 8424

_ax = np.arange(KS, dtype=np.float64) - (KS // 2)
_k1 = np.exp(-(_ax ** 2) / C2)
_ws = np.outer(_k1, _k1)
_ws = _ws / _ws.sum()
LOG_WS = np.log(_ws).astype(np.float32)   # [9, 9]

_CACHE = {}


def _build_nc(fillers_per_group=0, warmup_mms=10):
    """Build the single-core Bass program (SPMD across the 8 cores)."""
    from contextlib import ExitStack

    import concourse.bass as bass
    import concourse.tile as tile
    from concourse import bacc, mybir

    f32 = mybir.dt.float32
    bf16 = mybir.dt.bfloat16
    Alu = mybir.AluOpType
    Act = mybir.ActivationFunctionType

    class DedupBacc(bacc.Bacc):
        """Every matmul here uses the same identity stationary; drop the
        redundant per-matmul Ldweights the standard pipeline emits (the PE
        array keeps its weights between matmuls), moving their sem deps onto
        the following PE instruction before wait legalization."""

        def move_matmul_waits_to_ldweights(self):
            super().move_matmul_waits_to_ldweights()
            for bb in self.main_func.blocks:
                prev_key = None
                pending = None
                keep = []
                for ins in list(bb.instructions):
                    is_pe = getattr(ins, "engine", None) == self.tensor.engine
                    if isinstance(ins, mybir.InstLdweights):
                        key = str(ins.ins[0])
                        if key == prev_key:
                            pending = ins
                            continue
                        prev_key = key
                    if is_pe and pending is not None:
                        ins.merge_dependencies_from(pending)
                        pending = None
                    keep.append(ins)
                assert pending is None
                bb.instructions[:] = keep

    nc = DedupBacc("TRN2")
    xs_d = nc.dram_tensor("xs", [NPART, SLABF], bf16, kind="ExternalInput")
    xc_d = nc.dram_tensor("xc", [NPART, FREE], f32, kind="ExternalInput")
    bt_d = nc.dram_tensor("bt", [NPART, KS * KS], f32, kind="ExternalInput")
    id_d = nc.dram_tensor("ident", [NPART, NPART], bf16, kind="ExternalInput")
    y_d = nc.dram_tensor("y", [NPART, FREE], f32, kind="ExternalOutput")

    with ExitStack() as ctx:
        tc = ctx.enter_context(tile.TileContext(nc))
        singles = ctx.enter_context(tc.tile_pool(name="singles", bufs=1))
        tapp = ctx.enter_context(tc.tile_pool(name="tapp", bufs=2))
        psum = ctx.enter_context(tc.tile_pool(name="psum", bufs=1, space="PSUM"))
        fin = ctx.enter_context(tc.tile_pool(name="fin", bufs=1))

        slabA = singles.tile([NPART, SLABF], bf16)
        slabB = singles.tile([NPART, SLABF], bf16)
        xc_sb = singles.tile([NPART, FREE], f32)
        bt_sb = singles.tile([NPART, KS * KS], f32)
        id_sb = singles.tile([NPART, NPART], bf16)

        # PE HAM warmup: junk matmuls overlapped with the slab DMA so the
        # tensor engine is at full clock when the real matmuls start.
        junk = singles.tile([NPART, 512], bf16)
        psum_scr = psum.tile([NPART, 512], f32)
        nc.vector.memset(junk[:, :], 0)
        nc.sync.dma_start(out=id_sb[:, :], in_=id_d[:, :])
        # warmup loads the identity as PE stationary; every later matmul
        # reuses it (ldweights=False), eliminating per-matmul weight reloads
        for _ in range(warmup_mms):
            nc.tensor.matmul(psum_scr[:, :], id_sb[:, :], junk[:, :],
                             start=True, stop=True)

        # split big loads across DMA queues; xc is only needed by the tail
        HSL = SLABF // 2
        nc.sync.dma_start(out=slabA[:, 0:HSL], in_=xs_d[:, 0:HSL])
        nc.sync.dma_start(out=slabA[:, HSL:SLABF], in_=xs_d[:, HSL:SLABF])
        nc.sync.dma_start(out=bt_sb[:, :], in_=bt_d[:, :])
        # one-element-shifted copy for 4B-aligned odd-dj tap reads
        nc.sync.dma_start(out=slabB[:, 0:HSL], in_=xs_d[:, 1 : HSL + 1])
        nc.sync.dma_start(out=slabB[:, HSL : SLABF - 2], in_=xs_d[:, HSL + 1 : SLABF - 1])
        nc.sync.dma_start(out=xc_sb[:, :], in_=xc_d[:, :])

        sA = slabA[:, :].rearrange("p (g r c) -> p g r c", g=GROUPS, r=KS)
        center = sA[:, :, PAD, PAD : PAD + WSUB]  # [128, 9, 96]

        # center replicated across the (max 5) tap positions of a group so the
        # batched subtract has a plain non-broadcast second operand
        center5 = singles.tile([NPART, 5, GROUPS, WSUB], bf16)
        for t in range(5):
            nc.scalar.copy(center5[:, t, :, :], center)

        num0 = psum.tile([NPART, HALF], f32)
        num1 = psum.tile([NPART, HALF], f32)
        den0 = psum.tile([NPART, HALF], f32)
        den1 = psum.tile([NPART, HALF], f32)
        started = {0: False, 1: False, 2: False, 3: False}
        nbanks = (num0, num1, den0, den1)

        # interleave parities so DVE (even-group wd) and GPSIMD (odd-group
        # wd) stay concurrently busy; lead with two even groups so the
        # shifted slabB copy has time to land
        groups = []
        for di in range(KS):
            groups.append((di, 0))
            if di >= 2:
                groups.append((di - 2, 1))
        groups += [(KS - 2, 1), (KS - 1, 1)]
        n_groups = len(groups)

        for gi, (di, par) in enumerate(groups):
            djs = [dj for dj in range(KS) if dj % 2 == par]
            nt = len(djs)
            slab = slabA if par == 0 else slabB
            base = slab[:, :]
            p_ap = bass.AP(
                tensor=base.tensor,
                offset=base.offset + di * WPAD,
                ap=[list(base.ap[0]), [2, nt], [UNIT, GROUPS], [1, WSUB]],
            )

            d5 = tapp.tile([NPART, nt, GROUPS, WSUB], bf16, tag="d5", bufs=2)
            s5 = tapp.tile([NPART, nt, GROUPS, WSUB], bf16, tag="s5", bufs=2)
            w5 = tapp.tile([NPART, nt, GROUPS, WSUB], bf16, tag="w5", bufs=2)
            wd5 = tapp.tile([NPART, nt, GROUPS, WSUB], bf16, tag="wd5", bufs=4)

            nc.vector.tensor_tensor(
                d5[:, :, :, :], p_ap, center5[:, 0:nt, :, :], Alu.subtract)
            nc.vector.tensor_tensor(
                s5[:, :, :, :], d5[:, :, :, :], d5[:, :, :, :], Alu.mult)
            for k, dj in enumerate(djs):
                tcol = di * KS + dj
                nc.scalar.activation(
                    w5[:, k, :, :], s5[:, k, :, :], Act.Exp,
                    bias=bt_sb[:, tcol : tcol + 1], scale=-1.0 / C2,
                )
            eng = nc.gpsimd if par == 1 else nc.vector
            eng.tensor_tensor(
                wd5[:, :, :, :], w5[:, :, :, :], d5[:, :, :, :], Alu.mult)

            wfl = w5[:, :, :, :].rearrange("p t g c -> p (t g c)")
            wdfl = wd5[:, :, :, :].rearrange("p t g c -> p (t g c)")
            last_group = gi == n_groups - 1
            for b in range(2 * nt):
                half = b % 2
                cols = slice(b * HALF, (b + 1) * HALF)
                for bank_idx, rhs in ((half, wdfl[:, cols]),
                                      (2 + half, wfl[:, cols])):
                    tgt = nbanks[bank_idx]
                    mm = nc.tensor.matmul(
                        tgt[:, :], id_sb[:, :], rhs,
                        start=not started[bank_idx],
                        stop=last_group and b >= 2 * nt - 2,
                    )
                    mm.ins.ldweights = False
                    started[bank_idx] = True
            # keep the PE activity monitor from re-throttling the clock
            for _ in range(fillers_per_group):
                nc.tensor.matmul(psum_scr[:, :], junk[:, 0:NPART], junk[:, :],
                                 start=True, stop=True)

        y_sb = fin.tile([NPART, FREE], f32)
        for hb, (nm, dn) in enumerate(((num0, den0), (num1, den1))):
            r = fin.tile([NPART, HALF], f32, tag=f"r{hb}")
            scr = fin.tile([NPART, HALF], f32, tag=f"scr{hb}")
            nc.vector.reciprocal_approx_accurate(
                out=r[:, :], in_=dn[:, :], scratch=scr[:, :])
            t = fin.tile([NPART, HALF], f32, tag=f"t{hb}")
            nc.vector.tensor_tensor(t[:, :], nm[:, :], r[:, :], Alu.mult)
            nc.vector.tensor_tensor(
                y_sb[:, hb * HALF : (hb + 1) * HALF], t[:, :],
                xc_sb[:, hb * HALF : (hb + 1) * HALF], Alu.add)
        nc.sync.dma_start(out=y_d[:, :], in_=y_sb[:, :])

    nc.finalize()
    return nc


def get_nc():
    if "nc" not in _CACHE:
        _CACHE["nc"] = _build_nc()
    return _CACHE["nc"]


def host_shard(x):
    """x [B,C,H,W] f32 -> per-core dicts of device inputs."""
    xp = np.pad(x, ((0, 0), (0, 0), (PAD, PAD), (PAD, PAD)), mode="reflect")
    sw = np.lib.stride_tricks.sliding_window_view(xp, (KS, WPAD), axis=(2, 3))
    win = sw[:, :, :, ::WSUB]  # [B,C,384,4,9,104]
    bt = np.tile(LOG_WS.reshape(1, KS * KS), (NPART, 1)).astype(np.float32)
    ident = np.eye(NPART, dtype=BF16)
    in_maps = []
    for core in range(NCORES):
        h0 = core * HPER
        u = win[:, :, h0 : h0 + HPER].transpose(0, 1, 3, 2, 4, 5)
        slab = np.ascontiguousarray(u).reshape(NPART, SLABF).astype(BF16)
        xc = x[:, :, h0 : h0 + HPER].reshape(B, C, HPER, WQ, WSUB)
        xc = np.ascontiguousarray(xc.transpose(0, 1, 3, 2, 4))
        xc = xc.reshape(NPART, FREE).astype(np.float32)
        in_maps.append({"xs": slab, "xc": xc, "bt": bt, "ident": ident})
    return in_maps


def host_unshard(ys):
    out = np.empty((B, C, H, W), np.float32)
    for core in range(NCORES):
        h0 = core * HPER
        y = np.asarray(ys[core], np.float32).reshape(B, C, WQ, HPER, WSUB)
        out[:, :, h0 : h0 + HPER] = y.transpose(0, 1, 3, 2, 4).reshape(
            B, C, HPER, W)
    return out


def kernel(x, ksize):
    from concourse.bass_utils import run_bass_kernel_spmd

    assert int(ksize) == KS
    x = np.asarray(x, dtype=np.float32)
    assert x.shape == (B, C, H, W)
    in_maps = host_shard(x)
    nc = get_nc()
    res = run_bass_kernel_spmd(nc, in_maps, core_ids=list(range(NCORES)))
    ys = [np.asarray(r["y"]) for r in res.results]
    return host_unshard(ys)


# revision 11
# speedup vs baseline: 1.1264x; 1.0358x over previous
"""Bilateral filter (B,C,H,W)=(2,3,384,384), ksize=9 on 8 Trainium2 NeuronCores.

Strategy
--------
Data-parallel over H: core k owns output rows [48k, 48k+48) for every (b, c).

Host side packs, per core, 1152 "units" (one output row-segment of 96 pixels
each) into a [128 partitions x 9 groups] SBUF-friendly slab; each unit stores
its padded 9x104 input window (reflect padding resolved on host).  A tap
(di, dj) of the 9x9 stencil is then a pure free-dim offset read of the slab.

The 9x9 taps are processed in 18 groups (di x column-parity); one DVE
instruction covers all 5 (even dj) or 4 (odd dj) taps of a group through a
3-free-dim overlapped access pattern [(taps, step 2), (9 units, 936), (96, 1)],
amortizing the per-instruction + DRAIN overhead of the vector engine.

Per-tap math (the reference's per-pixel wd normalization cancels between
numerator and denominator):

    d   = p - x                      (DVE, bf16, batched per group)
    s   = d^2                        (DVE, bf16, batched per group)
    w   = exp(-s/(2 sigma^2) + ln ws_t)   (ACT, per-tap slice, free scale+bias)
    wd  = w * d                      (DVE for even groups, GPSIMD for odd)
    num += wd ; den += w             (TensorE identity-matmul into PSUM, fp32)

    out = x_f32 + num / den          (fp32 tail)

dtype: bf16 on-chip for 2x DVE tensor_tensor throughput; accumulation and
final arithmetic in fp32 (PSUM).  Odd-dj taps read a one-element-shifted slab
copy (slabB, its own DMA) so every DVE operand stays 4-byte aligned.

The tensor engine's HAM clock gate (1.2 vs 2.4 GHz) is kept warm with an
initial junk-matmul burst plus filler matmuls interleaved with the real
accumulation matmuls.
"""

import numpy as np
import ml_dtypes

BF16 = ml_dtypes.bfloat16

B, C, H, W = 2, 3, 384, 384
KS = 9
PAD = 4
SIGMA = 0.3 * ((KS - 1) / 2.0 - 1) + 0.8  # 1.7
C2 = 2.0 * SIGMA * SIGMA                  # 5.78
NCORES = 8
HPER = H // NCORES                        # 48
WQ = 4
WSUB = W // WQ                            # 96
WPAD = WSUB + 2 * PAD                     # 104
GROUPS = 9
NPART = 128
FREE = GROUPS * WSUB                      # 864
HALF = FREE // 2                          # 432
UNIT = KS * WPAD                          # 936
SLABF = GROUPS * UNIT                     # 8424

_ax = np.arange(KS, dtype=np.float64) - (KS // 2)
_k1 = np.exp(-(_ax ** 2) / C2)
_ws = np.outer(_k1, _k1)
_ws = _ws / _ws.sum()
LOG_WS = np.log(_ws).astype(np.float32)   # [9, 9]

_CACHE = {}


def _build_nc(fillers_per_group=0, warmup_mms=10):
    """Build the single-core Bass program (SPMD across the 8 cores)."""
    from contextlib import ExitStack

    import concourse.bass as bass
    import concourse.tile as tile
    from concourse import bacc, mybir

    f32 = mybir.dt.float32
    bf16 = mybir.dt.bfloat16
    Alu = mybir.AluOpType
    Act = mybir.ActivationFunctionType

    class DedupBacc(bacc.Bacc):
        """Every matmul here uses the same identity stationary; drop the
        redundant per-matmul Ldweights the standard pipeline emits (the PE
        array keeps its weights between matmuls), moving their sem deps onto
        the following PE instruction before wait legalization."""

        def move_matmul_waits_to_ldweights(self):
            super().move_matmul_waits_to_ldweights()
            for bb in self.main_func.blocks:
                prev_key = None
                pending = None
                keep = []
                for ins in list(bb.instructions):
                    is_pe = getattr(ins, "engine", None) == self.tensor.engine
                    if isinstance(ins, mybir.InstLdweights):
                        key = str(ins.ins[0])
                        if key == prev_key:
                            pending = ins
                            continue
                        prev_key = key
                    if is_pe and pending is not None:
                        ins.merge_dependencies_from(pending)
                        pending = None
                    keep.append(ins)
                assert pending is None
                bb.instructions[:] = keep

    nc = DedupBacc("TRN2")
    xs_d = nc.dram_tensor("xs", [NPART, SLABF], bf16, kind="ExternalInput")
    xc_d = nc.dram_tensor("xc", [NPART, FREE], f32, kind="ExternalInput")
    bt_d = nc.dram_tensor("bt", [NPART, KS * KS], f32, kind="ExternalInput")
    id_d = nc.dram_tensor("ident", [NPART, NPART], bf16, kind="ExternalInput")
    y_d = nc.dram_tensor("y", [NPART, FREE], f32, kind="ExternalOutput")

    with ExitStack() as ctx:
        tc = ctx.enter_context(tile.TileContext(nc))
        singles = ctx.enter_context(tc.tile_pool(name="singles", bufs=1))
        tapp = ctx.enter_context(tc.tile_pool(name="tapp", bufs=2))
        psum = ctx.enter_context(tc.tile_pool(name="psum", bufs=1, space="PSUM"))
        fin = ctx.enter_context(tc.tile_pool(name="fin", bufs=1))

        slabA = singles.tile([NPART, SLABF], bf16)
        slabB = singles.tile([NPART, SLABF], bf16)
        xc_sb = singles.tile([NPART, FREE], f32)
        bt_sb = singles.tile([NPART, KS * KS], f32)
        id_sb = singles.tile([NPART, NPART], bf16)

        # PE HAM warmup: junk matmuls overlapped with the slab DMA so the
        # tensor engine is at full clock when the real matmuls start.
        junk = singles.tile([NPART, 512], bf16)
        psum_scr = psum.tile([NPART, 512], f32)
        nc.vector.memset(junk[:, :], 0)
        nc.sync.dma_start(out=id_sb[:, :], in_=id_d[:, :])
        # warmup loads the identity as PE stationary; every later matmul
        # reuses it (ldweights=False), eliminating per-matmul weight reloads
        for _ in range(warmup_mms):
            nc.tensor.matmul(psum_scr[:, :], id_sb[:, :], junk[:, :],
                             start=True, stop=True)

        # split big loads across DMA queues; xc is only needed by the tail
        HSL = SLABF // 2
        nc.sync.dma_start(out=slabA[:, 0:HSL], in_=xs_d[:, 0:HSL])
        nc.sync.dma_start(out=slabA[:, HSL:SLABF], in_=xs_d[:, HSL:SLABF])
        nc.sync.dma_start(out=bt_sb[:, :], in_=bt_d[:, :])
        # one-element-shifted copy for 4B-aligned odd-dj tap reads
        nc.sync.dma_start(out=slabB[:, 0:HSL], in_=xs_d[:, 1 : HSL + 1])
        nc.sync.dma_start(out=slabB[:, HSL : SLABF - 2], in_=xs_d[:, HSL + 1 : SLABF - 1])
        nc.sync.dma_start(out=xc_sb[:, :], in_=xc_d[:, :])

        sA = slabA[:, :].rearrange("p (g r c) -> p g r c", g=GROUPS, r=KS)
        center = sA[:, :, PAD, PAD : PAD + WSUB]  # [128, 9, 96]

        # center replicated across the (max 5) tap positions of a group so the
        # batched subtract has a plain non-broadcast second operand
        center5 = singles.tile([NPART, 5, GROUPS, WSUB], bf16)
        for t in range(5):
            nc.scalar.copy(center5[:, t, :, :], center)

        num0 = psum.tile([NPART, HALF], f32)
        num1 = psum.tile([NPART, HALF], f32)
        den0 = psum.tile([NPART, HALF], f32)
        den1 = psum.tile([NPART, HALF], f32)
        started = {0: False, 1: False, 2: False, 3: False}
        nbanks = (num0, num1, den0, den1)

        # interleave parities so DVE (even-group wd) and GPSIMD (odd-group
        # wd) stay concurrently busy; lead with two even groups so the
        # shifted slabB copy has time to land
        groups = []
        for di in range(KS):
            groups.append((di, 0))
            if di >= 2:
                groups.append((di - 2, 1))
        groups += [(KS - 2, 1), (KS - 1, 1)]
        n_groups = len(groups)

        for gi, (di, par) in enumerate(groups):
            djs = [dj for dj in range(KS) if dj % 2 == par]
            nt = len(djs)
            slab = slabA if par == 0 else slabB
            base = slab[:, :]
            p_ap = bass.AP(
                tensor=base.tensor,
                offset=base.offset + di * WPAD,
                ap=[list(base.ap[0]), [2, nt], [UNIT, GROUPS], [1, WSUB]],
            )

            d5 = tapp.tile([NPART, nt, GROUPS, WSUB], bf16, tag="d5", bufs=3)
            s5 = tapp.tile([NPART, nt, GROUPS, WSUB], bf16, tag="s5", bufs=3)
            w5 = tapp.tile([NPART, nt, GROUPS, WSUB], bf16, tag="w5", bufs=3)
            wd5 = tapp.tile([NPART, nt, GROUPS, WSUB], bf16, tag="wd5", bufs=4)

            nc.vector.tensor_tensor(
                d5[:, :, :, :], p_ap, center5[:, 0:nt, :, :], Alu.subtract)
            nc.vector.tensor_tensor(
                s5[:, :, :, :], d5[:, :, :, :], d5[:, :, :, :], Alu.mult)
            for k, dj in enumerate(djs):
                tcol = di * KS + dj
                nc.scalar.activation(
                    w5[:, k, :, :], s5[:, k, :, :], Act.Exp,
                    bias=bt_sb[:, tcol : tcol + 1], scale=-1.0 / C2,
                )
            eng = nc.gpsimd if par == 1 else nc.vector
            eng.tensor_tensor(
                wd5[:, :, :, :], w5[:, :, :, :], d5[:, :, :, :], Alu.mult)

            wfl = w5[:, :, :, :].rearrange("p t g c -> p (t g c)")
            wdfl = wd5[:, :, :, :].rearrange("p t g c -> p (t g c)")
            last_group = gi == n_groups - 1
            for b in range(2 * nt):
                half = b % 2
                cols = slice(b * HALF, (b + 1) * HALF)
                for bank_idx, rhs in ((half, wdfl[:, cols]),
                                      (2 + half, wfl[:, cols])):
                    tgt = nbanks[bank_idx]
                    mm = nc.tensor.matmul(
                        tgt[:, :], id_sb[:, :], rhs,
                        start=not started[bank_idx],
                        stop=last_group and b >= 2 * nt - 2,
                    )
                    mm.ins.ldweights = False
                    started[bank_idx] = True
            # keep the PE activity monitor from re-throttling the clock
            for _ in range(fillers_per_group):
                nc.tensor.matmul(psum_scr[:, :], junk[:, 0:NPART], junk[:, :],
                                 start=True, stop=True)

        y_sb = fin.tile([NPART, FREE], f32)
        for hb, (nm, dn) in enumerate(((num0, den0), (num1, den1))):
            r = fin.tile([NPART, HALF], f32, tag=f"r{hb}")
            scr = fin.tile([NPART, HALF], f32, tag=f"scr{hb}")
            nc.vector.reciprocal_approx_accurate(
                out=r[:, :], in_=dn[:, :], scratch=scr[:, :])
            t = fin.tile([NPART, HALF], f32, tag=f"t{hb}")
            nc.vector.tensor_tensor(t[:, :], nm[:, :], r[:, :], Alu.mult)
            nc.vector.tensor_tensor(
                y_sb[:, hb * HALF : (hb + 1) * HALF], t[:, :],
                xc_sb[:, hb * HALF : (hb + 1) * HALF], Alu.add)
        nc.sync.dma_start(out=y_d[:, :], in_=y_sb[:, :])

    nc.finalize()
    return nc


def get_nc():
    if "nc" not in _CACHE:
        _CACHE["nc"] = _build_nc()
    return _CACHE["nc"]


def host_shard(x):
    """x [B,C,H,W] f32 -> per-core dicts of device inputs."""
    xp = np.pad(x, ((0, 0), (0, 0), (PAD, PAD), (PAD, PAD)), mode="reflect")
    sw = np.lib.stride_tricks.sliding_window_view(xp, (KS, WPAD), axis=(2, 3))
    win = sw[:, :, :, ::WSUB]  # [B,C,384,4,9,104]
    bt = np.tile(LOG_WS.reshape(1, KS * KS), (NPART, 1)).astype(np.float32)
    ident = np.eye(NPART, dtype=BF16)
    in_maps = []
    for core in range(NCORES):
        h0 = core * HPER
        u = win[:, :, h0 : h0 + HPER].transpose(0, 1, 3, 2, 4, 5)
        slab = np.ascontiguousarray(u).reshape(NPART, SLABF).astype(BF16)
        xc = x[:, :, h0 : h0 + HPER].reshape(B, C, HPER, WQ, WSUB)
        xc = np.ascontiguousarray(xc.transpose(0, 1, 3, 2, 4))
        xc = xc.reshape(NPART, FREE).astype(np.float32)
        in_maps.append({"xs": slab, "xc": xc, "bt": bt, "ident": ident})
    return in_maps


def host_unshard(ys):
    out = np.empty((B, C, H, W), np.float32)
    for core in range(NCORES):
        h0 = core * HPER
        y = np.asarray(ys[core], np.float32).reshape(B, C, WQ, HPER, WSUB)
        out[:, :, h0 : h0 + HPER] = y.transpose(0, 1, 3, 2, 4).reshape(
            B, C, HPER, W)
    return out


def kernel(x, ksize):
    from concourse.bass_utils import run_bass_kernel_spmd

    assert int(ksize) == KS
    x = np.asarray(x, dtype=np.float32)
    assert x.shape == (B, C, H, W)
    in_maps = host_shard(x)
    nc = get_nc()
    res = run_bass_kernel_spmd(nc, in_maps, core_ids=list(range(NCORES)))
    ys = [np.asarray(r["y"]) for r in res.results]
    return host_unshard(ys)
